# revision 2
# baseline (speedup 1.0000x reference)
"""Trainium2 Bass kernel for NaylisAttention (GQA + QK-RMSNorm + RoPE + low-rank
graph bias + causal softmax), tensor-parallel over heads across 8 NeuronCores.

Contract: kernel(**inputs) takes the FULL unsharded inputs (as produced by
reference.setup_inputs()) and returns the FULL [1, S, E] float32 output.

Sharding: 16 q-heads / 8 cores = 2 q-heads per core; each core owns the single
kv head its q-heads map to (GQA group of 4 -> kv head c//2). Each core computes
its partial out-projection y_c = attn_heads_c @ Wo[:, cols_c].T and the host
sums the 8 partials (the all-reduce / unshard step).

Device-side per core:
  - QKV projection with fp32r matmuls (full PE rate at N>=512 moving dim)
  - RMSNorm via sum-of-squares + rstd = exp(-0.5*ln(ms+eps)) (stays inside the
    single ACT 'exp' table; Sqrt would thrash the 1.3us table load)
  - RoPE with host-precomputed A/B tables that fold q_norm_w/k_norm_w and the
    1/sqrt(HD) score scale
  - causal flash attention; scores + low-rank bias accumulated into the same
    PSUM tile; exp on the scalar engine straight out of PSUM into bf16 P
    (no max-subtraction: scores are O(10), exp is safely in fp32 range)
  - P transposed on the PE in bf16; A@V in bf16 with a ones-column in V so the
    softmax denominator falls out of the same matmuls
  - normalization folded into the out_av PSUM->SBUF copy (per-partition scalar)
  - out-projection in fp32r
"""

import numpy as np
import ml_dtypes

import concourse.bacc as bacc
import concourse.tile as tile
from concourse import mybir

F32 = mybir.dt.float32
F32R = mybir.dt.float32r
BF16 = mybir.dt.bfloat16
AF = mybir.ActivationFunctionType

B, S, E = 1, 2048, 2048
H, NKV, R = 16, 4, 32
HD = E // H            # 128
N_CORES = 8
NB = S // 128          # 16 row blocks
NEC = E // 128         # 16 contraction chunks
NSC = S // 512         # 4 s-chunks
EPS = 1e-6
ROPE_BASE = 10000.0
NEG = -1e30


def build_nc():
    nc = bacc.Bacc(None, target_bir_lowering=False)

    xT = nc.dram_tensor("xT", [E, S], F32R, kind="ExternalInput")
    wqkv = nc.dram_tensor("wqkv", [E, 512], F32R, kind="ExternalInput")
    wrqk = nc.dram_tensor("wrqk", [E, 128], F32R, kind="ExternalInput")
    wo = nc.dram_tensor("wo", [256, E], F32R, kind="ExternalInput")
    tabs = nc.dram_tensor("tabs", [S, 768], F32R, kind="ExternalInput")
    negmask = nc.dram_tensor("negmask", [128, 128], F32, kind="ExternalInput")
    identr = nc.dram_tensor("identr", [128, 128], F32R, kind="ExternalInput")
    ident16 = nc.dram_tensor("ident16", [128, 128], BF16, kind="ExternalInput")
    y = nc.dram_tensor("y", [S, E], F32, kind="ExternalOutput")

    with tile.TileContext(nc) as tc:
        with tc.tile_pool(name="consts", bufs=1) as consts, \
             tc.tile_pool(name="resid", bufs=1) as resid, \
             tc.tile_pool(name="xp", bufs=18) as xp, \
             tc.tile_pool(name="tp", bufs=3) as tp, \
             tc.tile_pool(name="work", bufs=2) as work, \
             tc.tile_pool(name="ps", bufs=1, space="PSUM") as ps:

            # ---- resident constants ----
            wqkv_sb = consts.tile([128, NEC, 512], F32R)
            wrqk_sb = consts.tile([128, NEC, 128], F32R)
            wo_sb = consts.tile([128, 2, E], F32R)
            nm_sb = consts.tile([128, 128], F32)
            idr_sb = consts.tile([128, 128], F32R)
            id16_sb = consts.tile([128, 128], BF16)
            eps_sb = consts.tile([128, 1], F32)
            for c in range(NEC):
                nc.sync.dma_start(out=wqkv_sb[:, c, :], in_=wqkv[c * 128:(c + 1) * 128, :])
                nc.sync.dma_start(out=wrqk_sb[:, c, :], in_=wrqk[c * 128:(c + 1) * 128, :])
            for f in range(2):
                nc.sync.dma_start(out=wo_sb[:, f, :], in_=wo[f * 128:(f + 1) * 128, :])
            nc.sync.dma_start(out=nm_sb, in_=negmask[:, :])
            nc.sync.dma_start(out=idr_sb, in_=identr[:, :])
            nc.sync.dma_start(out=id16_sb, in_=ident16[:, :])
            nc.vector.memset(eps_sb, EPS)

            # ---- resident activations ----
            qT_sb = resid.tile([128, 2, S], F32R)      # [d, head, s]
            kT_sb = resid.tile([128, S], F32R)         # [d, s]
            rqkT_sb = resid.tile([128, S], F32R)       # rows 0:64 rq (h0,h1), 64:128 rk
            rk_sb = resid.tile([64, S], F32R)          # rk shifted to partitions 0:63
            v_sb = resid.tile([128, NB, 132], BF16)    # [j, jb, d + ones col]
            nc.vector.memset(v_sb, 0.0)
            nc.vector.memset(v_sb[:, :, 128:129], 1.0)

            # =========== phase 1: projections + norm + rope + transposes ===========
            for s4 in range(NSC):
                xt = []
                for c in range(NEC):
                    xtile = xp.tile([128, 512], F32R, tag="x", name=f"x_{s4}_{c}")
                    nc.sync.dma_start(out=xtile, in_=xT[c * 128:(c + 1) * 128, s4 * 512:(s4 + 1) * 512])
                    xt.append(xtile)
                tabt = []
                for il in range(4):
                    i = s4 * 4 + il
                    tt = tp.tile([128, 768], F32R, tag="tab", name=f"tab_{i}")
                    nc.sync.dma_start(out=tt, in_=tabs[i * 128:(i + 1) * 128, :])
                    tabt.append(tt)

                # rq/rk projection: psum rows 0:64 = rq(h0,h1), 64:128 = rk(h0,h1)
                prq = ps.tile([128, 512], F32, tag="rqkT", bufs=1, name=f"prq_{s4}")
                for c in range(NEC):
                    nc.tensor.matmul(prq[:, :], wrqk_sb[:, c, :], xt[c][:, :],
                                     start=(c == 0), stop=(c == NEC - 1))
                nc.scalar.activation(rqkT_sb[:, s4 * 512:(s4 + 1) * 512], prq[:, :], AF.Copy)
                # shift rk rows (64:128) down to partitions 0:63 via SBUF->SBUF DMA
                nc.sync.dma_start(out=rk_sb[0:64, s4 * 512:(s4 + 1) * 512],
                                  in_=rqkT_sb[64:128, s4 * 512:(s4 + 1) * 512])

                for il in range(4):
                    i = s4 * 4 + il
                    # qkv projection for row block i: psum [128, 512] = [q0|q1|k|v]
                    pqkv = ps.tile([128, 512], F32, tag="big", bufs=3, name=f"pqkv_{i}")
                    for c in range(NEC):
                        nc.tensor.matmul(pqkv[:, :], xt[c][:, il * 128:(il + 1) * 128],
                                         wqkv_sb[:, c, :], start=(c == 0), stop=(c == NEC - 1))

                    # rmsnorm: ms = mean(q^2) per head; rstd = exp(-0.5*ln(ms+eps))
                    # Square on ACT with accum_out gives the row sum directly
                    # (PSUM may only feed one non-scalar DVE input, so no DVE mul)
                    sq = work.tile([128, 128], F32, tag="sq", name=f"sq_{i}")
                    ssum = work.tile([128, 3], F32, tag="ssum", name=f"ssum_{i}")
                    for g in range(3):
                        nc.scalar.activation(sq[:, :], pqkv[:, g * 128:(g + 1) * 128],
                                             AF.Square, accum_out=ssum[:, g:g + 1])
                    lnms = work.tile([128, 3], F32, tag="lnms", name=f"lnms_{i}")
                    nc.scalar.activation(lnms[:, :], ssum[:, :], AF.Ln, bias=eps_sb[:, :], scale=1.0 / 128.0)
                    rstd = work.tile([128, 3], F32, tag="rstd", name=f"rstd_{i}")
                    nc.scalar.activation(rstd[:, :], lnms[:, :], AF.Exp, scale=-0.5)

                    # copy q0|q1|k out of psum with rstd folded in; v to bf16
                    qk = work.tile([128, 384], F32R, tag="qk", name=f"qk_{i}")
                    for g in range(3):
                        nc.scalar.activation(qk[:, g * 128:(g + 1) * 128],
                                             pqkv[:, g * 128:(g + 1) * 128],
                                             AF.Copy, scale=rstd[:, g:g + 1])
                    nc.scalar.activation(v_sb[:, i, 0:128], pqkv[:, 384:512], AF.Copy)

                    # rope: roped = qk*A + shuffle(qk)*B  (A/B fold w, cos/sin, scale)
                    tt = tabt[il]
                    roped = work.tile([128, 384], F32R, tag="roped", name=f"roped_{i}")
                    nc.vector.tensor_mul(roped[:, :], qk[:, :], tt[:, 0:384])
                    tb = work.tile([128, 384], F32R, tag="tb", name=f"tb_{i}")
                    qk3 = qk[:, :].rearrange("p (g t e) -> p g t e", g=3, t=2)
                    tb3 = tb[:, :].rearrange("p (g t e) -> p g t e", g=3, t=2)
                    B3 = tt[:, 384:768].rearrange("p (g t e) -> p g t e", g=3, t=2)
                    nc.vector.tensor_mul(tb3[:, :, 0, :], qk3[:, :, 1, :], B3[:, :, 0, :])
                    nc.vector.tensor_mul(tb3[:, :, 1, :], qk3[:, :, 0, :], B3[:, :, 1, :])
                    nc.vector.tensor_add(roped[:, :], roped[:, :], tb[:, :])

                    # transpose q0,q1 -> qT, k -> kT
                    for g in range(3):
                        ptr = ps.tile([128, 128], F32R, tag="pt", bufs=2, name=f"ptr_{i}_{g}")
                        nc.tensor.transpose(ptr[:, :], roped[:, g * 128:(g + 1) * 128], idr_sb[:, :])
                        if g < 2:
                            dst = qT_sb[:, g, i * 128:(i + 1) * 128]
                        else:
                            dst = kT_sb[:, i * 128:(i + 1) * 128]
                        nc.scalar.activation(dst, ptr[:, :], AF.Copy)

            # =========== phase 2: attention + out-projection per row block ===========
            for i in range(NB):
                nchunks = i // 4 + 1
                wlast = (i + 1) * 128 - (nchunks - 1) * 512
                p_tiles = []
                for h in range(2):
                    P_sb = work.tile([128, S], BF16, tag="P", name=f"P_{i}_{h}")
                    p_tiles.append(P_sb)
                    for ch in range(nchunks):
                        w = 512 if ch < nchunks - 1 else wlast
                        pss = ps.tile([128, 512], F32, tag="big", bufs=3, name=f"ps_{i}_{h}_{ch}")
                        nc.tensor.matmul(pss[:, 0:w], qT_sb[:, h, i * 128:(i + 1) * 128],
                                         kT_sb[:, ch * 512:ch * 512 + w], start=True, stop=False)
                        nc.tensor.matmul(pss[:, 0:w], rqkT_sb[32 * h:32 * h + 32, i * 128:(i + 1) * 128],
                                         rk_sb[32 * h:32 * h + 32, ch * 512:ch * 512 + w],
                                         start=False, stop=True)
                        if ch == nchunks - 1:
                            off = w - 128
                            nc.vector.tensor_add(pss[:, off:off + 128], pss[:, off:off + 128], nm_sb[:, :])
                        nc.scalar.activation(P_sb[:, ch * 512:ch * 512 + w], pss[:, 0:w], AF.Exp)

                oav = work.tile([128, 2, 128], F32R, tag="oav", name=f"oav_{i}")
                for h in range(2):
                    P_sb = p_tiles[h]
                    pav = ps.tile([128, 132], F32, tag="av", bufs=2, name=f"pav_{i}_{h}")
                    for grp in range((i + 4) // 4):
                        used = min(4, i + 1 - grp * 4)
                        ptp = ps.tile([128, 512], BF16, tag="pt", bufs=2, name=f"ptp_{i}_{h}_{grp}")
                        for q in range(used):
                            jb = grp * 4 + q
                            nc.tensor.transpose(ptp[:, q * 128:(q + 1) * 128],
                                                P_sb[:, jb * 128:(jb + 1) * 128], id16_sb[:, :])
                        pts = work.tile([128, 512], BF16, tag="pts", name=f"pts_{i}_{h}_{grp}")
                        nc.vector.tensor_copy(pts[:, 0:used * 128], ptp[:, 0:used * 128])
                        for q in range(used):
                            jb = grp * 4 + q
                            nc.tensor.matmul(pav[:, :], pts[:, q * 128:(q + 1) * 128],
                                             v_sb[:, jb, :], start=(jb == 0), stop=(jb == i),
                                             skip_group_check=True)
                    rsum = work.tile([128, 1], F32, tag="rsum", name=f"rsum_{i}_{h}")
                    nc.vector.reciprocal(rsum[:, :], pav[:, 128:129])
                    nc.vector.tensor_scalar_mul(oav[:, h, :], pav[:, 0:128], rsum[:, 0:1])

                # transpose out_av -> [f, i] and out-project
                oavT = work.tile([128, 2, 128], F32R, tag="oavT", name=f"oavT_{i}")
                for h in range(2):
                    ptr = ps.tile([128, 128], F32R, tag="pt", bufs=2, name=f"ptro_{i}_{h}")
                    nc.tensor.transpose(ptr[:, :], oav[:, h, :], idr_sb[:, :])
                    nc.scalar.activation(oavT[:, h, :], ptr[:, :], AF.Copy)

                y_sb = work.tile([128, E], F32, tag="y", name=f"y_{i}")
                for ec in range(4):
                    py = ps.tile([128, 512], F32, tag="big", bufs=3, name=f"py_{i}_{ec}")
                    nc.tensor.matmul(py[:, :], oavT[:, 0, :], wo_sb[:, 0, ec * 512:(ec + 1) * 512],
                                     start=True, stop=False)
                    nc.tensor.matmul(py[:, :], oavT[:, 1, :], wo_sb[:, 1, ec * 512:(ec + 1) * 512],
                                     start=False, stop=True)
                    nc.vector.tensor_copy(y_sb[:, ec * 512:(ec + 1) * 512], py[:, :])
                nc.sync.dma_start(out=y[i * 128:(i + 1) * 128, :], in_=y_sb[:, :])

    return nc


def make_core_inputs(inputs, core):
    """Host-side sharding/prep for one core. Returns dict keyed by BIR names."""
    x = np.asarray(inputs["x"], dtype=np.float32)
    Wq = np.asarray(inputs["Wq"], dtype=np.float32)
    Wk = np.asarray(inputs["Wk"], dtype=np.float32)
    Wv = np.asarray(inputs["Wv"], dtype=np.float32)
    Wo = np.asarray(inputs["Wo"], dtype=np.float32)
    Wrq = np.asarray(inputs["Wrq"], dtype=np.float32)
    Wrk = np.asarray(inputs["Wrk"], dtype=np.float32)
    gs = np.asarray(inputs["graph_scale"], dtype=np.float32)
    qw = np.asarray(inputs["q_norm_w"], dtype=np.float32)
    kw = np.asarray(inputs["k_norm_w"], dtype=np.float32)

    c = core
    h0 = 2 * c
    g = c // 2
    scale = np.float32(1.0 / np.sqrt(HD))

    wqkv = np.concatenate([
        Wq[h0 * 128:(h0 + 2) * 128],       # 256 rows
        Wk[g * 128:(g + 1) * 128],         # 128
        Wv[g * 128:(g + 1) * 128],         # 128
    ], axis=0).T.copy()                    # [E, 512]
    wrqk = np.concatenate([
        gs[h0] * Wrq[h0 * 32:(h0 + 1) * 32],
        gs[h0 + 1] * Wrq[(h0 + 1) * 32:(h0 + 2) * 32],
        Wrk[h0 * 32:(h0 + 1) * 32],
        Wrk[(h0 + 1) * 32:(h0 + 2) * 32],
    ], axis=0).T.copy()                    # [E, 128]
    wo_c = Wo[:, h0 * 128:(h0 + 2) * 128].T.copy()   # [256, E]

    return {
        "wqkv": np.ascontiguousarray(wqkv),
        "wrqk": np.ascontiguousarray(wrqk),
        "wo": np.ascontiguousarray(wo_c),
    }


def make_shared_inputs(inputs):
    x = np.asarray(inputs["x"], dtype=np.float32)
    qw = np.asarray(inputs["q_norm_w"], dtype=np.float32)
    kw = np.asarray(inputs["k_norm_w"], dtype=np.float32)
    scale = np.float32(1.0 / np.sqrt(HD))

    xT = np.ascontiguousarray(x[0].T)

    inv = (1.0 / (ROPE_BASE ** (np.arange(0, HD, 2, dtype=np.float32) / HD))).astype(np.float32)
    t = np.arange(S, dtype=np.float32)
    fr = np.outer(t, inv).astype(np.float32)          # [S, 64]
    emb = np.concatenate([fr, fr], axis=1)            # [S, 128]
    cos = np.cos(emb).astype(np.float32)
    sin = np.sin(emb).astype(np.float32)

    dd = np.arange(128)
    sgn = np.where(dd < 64, -1.0, 1.0).astype(np.float32)
    A = np.empty((S, 384), np.float32)
    Bt = np.empty((S, 384), np.float32)
    for grp in range(3):
        w = qw if grp < 2 else kw
        s = scale if grp < 2 else np.float32(1.0)
        A[:, grp * 128:(grp + 1) * 128] = cos * (w * s)[None, :]
        Bt[:, grp * 128:(grp + 1) * 128] = sin * (sgn * w * s)[None, :]
    tabs = np.concatenate([A, Bt], axis=1)            # [S, 768]

    jj, ii = np.meshgrid(np.arange(128), np.arange(128))
    negmask = np.where(jj <= ii, 0.0, NEG).astype(np.float32)
    identr = np.eye(128, dtype=np.float32)
    ident16 = np.eye(128, dtype=ml_dtypes.bfloat16)

    return {"xT": xT, "tabs": np.ascontiguousarray(tabs), "negmask": negmask,
            "identr": identr, "ident16": ident16}


_NC_CACHE = None


def _get_nc():
    global _NC_CACHE
    if _NC_CACHE is None:
        nc = build_nc()
        nc.finalize()
        _NC_CACHE = nc
    return _NC_CACHE


def kernel(**inputs):
    from concourse.bass_utils import run_bass_kernel_spmd

    nc = _get_nc()
    shared = make_shared_inputs(inputs)
    in_maps = []
    for c in range(N_CORES):
        m = dict(shared)
        m.update(make_core_inputs(inputs, c))
        in_maps.append(m)
    res = run_bass_kernel_spmd(nc, in_maps, list(range(N_CORES)))
    acc = np.zeros((S, E), dtype=np.float64)
    for r in res.results:
        acc += r["y"].astype(np.float64)
    return acc.astype(np.float32).reshape(B, S, E)
